# revision 1
# baseline (speedup 1.0000x reference)
"""Multi-head causal attention (B=4, T=2048, D=1024, H=16, HS=64) on 8 TRN2
NeuronCores.

Sharding: batch (4-way) x head-group (2-way).  Core c handles batch c//2 and
heads 8*(c%2) .. 8*(c%2)+7.  Each core computes its 8 heads' attention and the
partial output projection Y_T = sum_h Wo_h^T @ O_T_h; the host sums the two
head-group partials per batch, transposes, and adds the output bias.

Per-core program (all matmuls contract along the partition dim; matmul
datapath in bf16 with fp32 PSUM accumulation, softmax denominator in fp32):
  - x (bf16) is PE-transposed into x^T [d, t]; V^T/Q^T/K^T [e2, t] come from
    matmul(lhsT=W[d, e2], rhs=x^T) with head pairs packed on the PE M axis;
    V^T is re-transposed into V_aug [k, 65] (ones column -> the softmax
    denominator accumulates inside the attn@v matmul for free).
  - S^T blocks [k=128, q=512] = matmul(lhsT=K^T, rhs=Q^T); exp on ScalarE
    (1/sqrt(HS) folded into the activation scale; no max subtraction --
    |scores| <= ~6 so exp cannot overflow); causal mask = upper-tri 0/1
    multiply on the one diagonal sub-block + column offsets on attn@v.
  - O^T_aug [65, q] accumulates over k chunks in PSUM; normalization uses a
    DRAM-bounced partition-broadcast of 1/l (reciprocal_approx_fast).
  - Output projection Y^T[d,q] = sum_pairs matmul(lhsT=Wo[e2,d], rhs=O^T);
    pairs 0-2 are pre-accumulated to SBUF during pair-3's attention.

Engine-level scheduling: Trainium engines execute in order, so emission order
is the schedule.  S^T runs two chunk-pairs ahead of attn@v, and independent
PE work (next head-pair's Q/K projections, out-projection chunks) is emitted
as filler inside the attention stream -- this keeps the PE busy enough that
the HAM clock gate stays at 2.4 GHz instead of demoting to 1.2 GHz.
"""

import numpy as np

B, T, D = 4, 2048, 1024
H, HS = 16, 64
NCORES = 8
NPAIR = 4   # head pairs per core
ND = 8      # 128-wide d chunks
NT = 16     # 128-wide t chunks
NQ = 4      # 512-wide q chunks
NK = 16     # 128-wide k chunks

_CACHE = {}


def _build_program():
    import concourse.bass as bass
    import concourse.tile as tile
    from concourse import bacc, mybir
    from contextlib import ExitStack

    f32 = mybir.dt.float32
    f32r = mybir.dt.float32r
    bf16 = mybir.dt.bfloat16
    Exp = mybir.ActivationFunctionType.Exp

    nc = bacc.Bacc("TRN2", target_bir_lowering=False, debug=False)

    x_d = nc.declare_dram_parameter("x", [128, NQ, ND, 512], bf16, isOutput=False)
    wq_d = nc.declare_dram_parameter("wq", [NPAIR, 128, ND, 128], bf16, isOutput=False)
    wk_d = nc.declare_dram_parameter("wk", [NPAIR, 128, ND, 128], bf16, isOutput=False)
    wv_d = nc.declare_dram_parameter("wv", [NPAIR, 128, ND, 128], bf16, isOutput=False)
    wo_d = nc.declare_dram_parameter("wo", [128, NPAIR, ND, 128], bf16, isOutput=False)
    tri_d = nc.declare_dram_parameter("tri", [128, 128], bf16, isOutput=False)
    idn_d = nc.declare_dram_parameter("ident", [128, 128], bf16, isOutput=False)
    yt_d = nc.declare_dram_parameter("yt", [D, T], f32, isOutput=True)

    with tile.TileContext(nc) as tc, ExitStack() as top:
        const = top.enter_context(tc.tile_pool(name="const", bufs=1))
        ident_sb = const.tile([128, 128], bf16, name="ident_sb")
        nc.sync.dma_start(out=ident_sb, in_=idn_d[:, :])
        tri_sb = const.tile([128, 128], bf16, name="tri_sb")
        nc.sync.dma_start(out=tri_sb, in_=tri_d[:, :])

        big = top.enter_context(tc.tile_pool(name="big", bufs=1))
        vaug = big.tile([128, 2 * NPAIR, NK, 65], bf16, name="vaug")
        nc.vector.memset(vaug[:, :, :, 64:65], 1.0)

        # PSUM banks: mm 3 + S 2*2 + O 1 = 8
        psM = top.enter_context(tc.tile_pool(name="psM", bufs=3, space="PSUM"))
        psS = top.enter_context(tc.tile_pool(name="psS", bufs=2, space="PSUM"))
        psO = top.enter_context(tc.tile_pool(name="psO", bufs=1, space="PSUM"))
        pw = top.enter_context(tc.tile_pool(name="pw", bufs=2))
        qkp = top.enter_context(tc.tile_pool(name="qkp", bufs=2))
        otn_p = top.enter_context(tc.tile_pool(name="otn_p", bufs=1))
        otn = otn_p.tile([128, NPAIR, T], bf16, name="otn")
        ptp = top.enter_context(tc.tile_pool(name="ptp", bufs=4))
        ocp = top.enter_context(tc.tile_pool(name="ocp", bufs=2))
        rcp = top.enter_context(tc.tile_pool(name="rcp", bufs=2))
        lbp = top.enter_context(tc.tile_pool(name="lbp", bufs=2))
        drp = top.enter_context(tc.tile_pool(name="drp", bufs=4, space="DRAM"))

        def dma_w(wdram, p, kind, pool=None):
            pool = pool or pw
            w_sb = pool.tile([128, ND, 128], bf16, tag="w", name=f"w_{kind}{p}")
            nc.sync.dma_start(out=w_sb, in_=wdram[p])
            return w_sb

        def attn_group(p, hh, j, qt, kt, filler):
            """One (head, q-chunk) attention group with pipelined emission."""
            h = 2 * p + hh
            e0 = hh * 64
            po = psO.tile([65, 512], f32, tag="O", name="po")
            ncc = 4 * (j + 1)
            nm = ncc // 2
            pts = {}

            def off_of(c):
                sub = c - 4 * j
                return sub * 128 if 0 <= sub < 4 else 0

            def emit_s(m):
                ps = psS.tile([128, 2, 512], f32, tag="S", name="ps")
                pt = ptp.tile([128, 2, 512], bf16, tag="pt", name="pt")
                for i in range(2):
                    c = 2 * m + i
                    off = off_of(c)
                    nc.tensor.matmul(
                        ps[:, i, off:],
                        kt[e0:e0 + 64, c * 128:(c + 1) * 128],
                        qt[e0:e0 + 64, j * 512 + off:(j + 1) * 512],
                        start=True,
                        stop=True,
                    )
                o0, o1 = off_of(2 * m), off_of(2 * m + 1)
                if o0 == o1:
                    nc.scalar.activation(out=pt[:, :, o0:], in_=ps[:, :, o0:],
                                         func=Exp, scale=0.125)
                else:
                    for i, off in ((0, o0), (1, o1)):
                        nc.scalar.activation(out=pt[:, i, off:],
                                             in_=ps[:, i, off:],
                                             func=Exp, scale=0.125)
                for i in range(2):
                    c = 2 * m + i
                    sub = c - 4 * j
                    if 0 <= sub < 4:
                        nc.vector.tensor_mul(
                            pt[:, i, sub * 128:(sub + 1) * 128],
                            pt[:, i, sub * 128:(sub + 1) * 128],
                            tri_sb,
                        )
                pts[m] = pt

            def emit_v(m):
                pt = pts.pop(m)
                for i in range(2):
                    c = 2 * m + i
                    off = off_of(c)
                    nc.tensor.matmul(
                        po[:, off:],
                        vaug[:, h, c, :],
                        pt[:, i, off:],
                        start=(c == 0),
                        stop=(c == ncc - 1),
                    )

            emit_s(0)
            if nm > 1:
                emit_s(1)
            for m in range(nm):
                if m + 2 < nm:
                    emit_s(m + 2)
                filler()
                emit_v(m)

            # normalize: otn[e, q] = O_T[e, q] / l[q]
            oc = ocp.tile([64, 512], f32, tag="oc", name="oc")
            nc.vector.tensor_copy(out=oc, in_=po[0:64, :])
            rl = rcp.tile([1, 512], f32, tag="rl", name="rl")
            nc.vector.tensor_copy(out=rl, in_=po[64:65, :])
            rd = drp.tile([1, 512], f32, tag="rd", name="rd")
            nc.sync.dma_start(out=rd, in_=rl)
            lb = lbp.tile([64, 512], f32, tag="lb", name="lb")
            nc.sync.dma_start(out=lb, in_=rd[0:1, :].partition_broadcast(64))
            nc.vector.reciprocal_approx_fast(lb, lb)
            nc.vector.tensor_mul(
                otn[e0:e0 + 64, p, j * 512:(j + 1) * 512], oc, lb
            )

        with ExitStack() as mid:
            xtp = mid.enter_context(tc.tile_pool(name="xtp", bufs=1))
            xt = xtp.tile([128, NQ, ND, 512], bf16, name="xt")

            def proj_mms(ps_t4, w_sb, t4, dc_lo, dc_hi):
                for dc in range(dc_lo, dc_hi):
                    nc.tensor.matmul(
                        ps_t4,
                        w_sb[:, dc, :],
                        xt[:, t4, dc, :],
                        start=(dc == 0),
                        stop=(dc == ND - 1),
                    )

            def proj_copy(dest_tile, ps_t4, t4, act=False):
                dst = dest_tile[:, t4 * 512:(t4 + 1) * 512]
                if act:
                    nc.scalar.copy(out=dst, in_=ps_t4)
                else:
                    nc.vector.tensor_copy(out=dst, in_=ps_t4)

            # ---- Phase A: x^T / V-proj / V-transpose, DMA-overlapped -------
            with ExitStack() as ph:
                vts = ph.enter_context(tc.tile_pool(name="vts", bufs=3))
                pwv = ph.enter_context(tc.tile_pool(name="pwv", bufs=4))

                wv_sbs = [None] * NPAIR
                vstash = {}

                def emit_vproj(pv):
                    t4, p = pv // 4, pv % 4
                    ps_t4 = psM.tile([128, 512], f32, tag="mm", name="psv")
                    proj_mms(ps_t4, wv_sbs[p], t4, 0, ND)
                    vt = vts.tile([128, 512], bf16, tag="vt", name="vt")
                    nc.scalar.copy(out=vt, in_=ps_t4)
                    vstash[pv] = vt

                def emit_vtr(pv):
                    t4, p = pv // 4, pv % 4
                    vt = vstash.pop(pv)
                    for hh in range(2):
                        for cl2 in range(2):
                            ptr = psS.tile([128, 2, 1024], bf16, tag="S",
                                           name="ptr_v")
                            for i in range(2):
                                cl = 2 * cl2 + i
                                nc.tensor.transpose(
                                    ptr[:, i, 0:64],
                                    vt[hh * 64:hh * 64 + 64,
                                       cl * 128:(cl + 1) * 128],
                                    ident_sb[hh * 64:hh * 64 + 64,
                                             hh * 64:hh * 64 + 64],
                                )
                            c = 4 * t4 + 2 * cl2
                            nc.vector.tensor_copy(
                                out=vaug[:, 2 * p + hh, c:c + 2, 0:64],
                                in_=ptr[:, :, 0:64],
                            )

                for t4 in range(NQ):
                    nc.sync.dma_start(out=xt[:, t4, :, :], in_=x_d[:, t4, :, :])
                    wv_sbs[t4] = dma_w(wv_d, t4, "v", pool=pwv)
                for pv in range(4 * NPAIR):
                    emit_vproj(pv)
                    if pv >= 1:
                        emit_vtr(pv - 1)
                emit_vtr(4 * NPAIR - 1)

                qt0 = qkp.tile([128, T], bf16, tag="qt", name="qt0")
                kt0 = qkp.tile([128, T], bf16, tag="kt", name="kt0")
                for w_d_, dest, kind in ((wq_d, qt0, "q"), (wk_d, kt0, "k")):
                    w_sb = dma_w(w_d_, 0, kind)
                    for t4 in range(NQ):
                        ps_t4 = psM.tile([128, 512], f32, tag="mm", name="psqk")
                        proj_mms(ps_t4, w_sb, t4, 0, ND)
                        proj_copy(dest, ps_t4, t4, act=True)

            # ---- Phase B, pairs 0-2: attention + next-pair Q/K filler ------
            qt_cur, kt_cur = qt0, kt0
            for p in range(NPAIR - 1):
                fill = []
                qt_nxt = qkp.tile([128, T], bf16, tag="qt", name=f"qt{p+1}")
                kt_nxt = qkp.tile([128, T], bf16, tag="kt", name=f"kt{p+1}")
                wq_nxt = dma_w(wq_d, p + 1, "q")
                wk_nxt = dma_w(wk_d, p + 1, "k")
                state = {"ps": None}

                def mk_unit(w_sb, dest, t4, dc_lo, dc_hi, state=state):
                    def emit():
                        if dc_lo == 0:
                            state["ps"] = psM.tile([128, 512], f32, tag="mm",
                                                   name="psf")
                        proj_mms(state["ps"], w_sb, t4, dc_lo, dc_hi)
                        if dc_hi == ND:
                            proj_copy(dest, state["ps"], t4)
                    return emit

                for w_sb, dest in ((wq_nxt, qt_nxt), (wk_nxt, kt_nxt)):
                    for t4 in range(NQ):
                        for dc_lo in range(0, ND, 4):
                            fill.append(mk_unit(w_sb, dest, t4, dc_lo,
                                                dc_lo + 4))

                def filler(fill=fill):
                    if fill:
                        fill.pop(0)()

                for hh in range(2):
                    for j in range(NQ):
                        attn_group(p, hh, j, qt_cur, kt_cur, filler)
                while fill:
                    fill.pop(0)()
                qt_cur, kt_cur = qt_nxt, kt_nxt

        # ---- Tail: pair 3 attention + output projection --------------------
        # head 6 filler: partial out-proj over pairs 0-2 (staged to SBUF);
        # head 7 filler: pair-3 contribution + combine, lagging 2 q-chunks.
        prt_p = top.enter_context(tc.tile_pool(name="prt_p", bufs=1))
        prt = prt_p.tile([128, NQ, ND, 512], f32, name="prt")
        pwo = top.enter_context(tc.tile_pool(name="pwo", bufs=1))
        pyt = top.enter_context(tc.tile_pool(name="pyt", bufs=3))
        wo_sb = pwo.tile([128, NPAIR, ND, 128], bf16, name="wo_sb")
        nc.sync.dma_start(out=wo_sb, in_=wo_d[:, :, :, :])

        def partial_unit(dc, qc):
            def emit():
                py = psM.tile([128, 512], f32, tag="mm", name="pyp")
                for pp in range(NPAIR - 1):
                    nc.tensor.matmul(
                        py,
                        wo_sb[:, pp, dc, :],
                        otn[:, pp, qc * 512:(qc + 1) * 512],
                        start=(pp == 0),
                        stop=(pp == NPAIR - 2),
                    )
                nc.vector.tensor_copy(out=prt[:, qc, dc, :], in_=py)
            return emit

        def final_unit(dc, qc):
            def emit():
                py = psM.tile([128, 512], f32, tag="mm", name="pyf")
                nc.tensor.matmul(
                    py,
                    wo_sb[:, 3, dc, :],
                    otn[:, 3, qc * 512:(qc + 1) * 512],
                    start=True,
                    stop=True,
                )
                yt_sb = pyt.tile([128, 512], f32, tag="yt", name="yt_f")
                nc.vector.tensor_add(yt_sb, prt[:, qc, dc, :], py)
                nc.sync.dma_start(
                    out=yt_d[dc * 128:(dc + 1) * 128,
                             qc * 512:(qc + 1) * 512],
                    in_=yt_sb,
                )
            return emit

        fill3 = [partial_unit(dc, qc) for qc in range(NQ) for dc in range(ND)]
        ffin = []
        done = set()

        def filler3(fill3=fill3):
            if fill3:
                fill3.pop(0)()

        def filler7():
            if ffin:
                ffin.pop(0)()
            elif fill3:
                fill3.pop(0)()

        for j in range(NQ):
            attn_group(3, 0, j, qt_cur, kt_cur, filler3)
        for j in range(NQ):
            if j >= 2:
                qc = j - 2
                for dc in range(ND):
                    ffin.append(final_unit(dc, qc))
                    done.add((dc, qc))
            attn_group(3, 1, j, qt_cur, kt_cur, filler7)
        while fill3:
            fill3.pop(0)()
        while ffin:
            ffin.pop(0)()
        for qc in range(NQ):
            for dc in range(ND):
                if (dc, qc) not in done:
                    final_unit(dc, qc)()

    nc.compile()
    return nc


def _pack_inputs(x, Wq, Wk, Wv, Wo):
    """Per-core input maps. Core c: batch c//2, head group c%2."""
    import ml_dtypes

    tri = np.triu(np.ones((128, 128), np.float32)).astype(ml_dtypes.bfloat16)
    ident = np.eye(128, dtype=np.float32).astype(ml_dtypes.bfloat16)

    def pack_w(W, g):
        # [NPAIR, 128(d_local), ND, 128(e2)]
        out = np.empty((NPAIR, 128, ND, 128), np.float32)
        for p in range(NPAIR):
            h1 = 8 * g + 2 * p
            r = W[[h1, h1 + 1]].transpose(1, 0, 2).reshape(D, 128)  # [d, e2]
            out[p] = r.reshape(ND, 128, 128).transpose(1, 0, 2)
        return np.ascontiguousarray(out).astype(ml_dtypes.bfloat16)

    def pack_wo(Wo, g):
        # [128(e2), NPAIR, ND, 128(d)]
        out = np.empty((128, NPAIR, ND, 128), np.float32)
        for p in range(NPAIR):
            r0 = (8 * g + 2 * p) * 64
            out[:, p] = Wo[r0:r0 + 128].reshape(128, ND, 128)
        return np.ascontiguousarray(out).astype(ml_dtypes.bfloat16)

    packs = {}
    for g in range(2):
        packs[g] = dict(
            wq=pack_w(Wq, g), wk=pack_w(Wk, g), wv=pack_w(Wv, g),
            wo=pack_wo(Wo, g),
        )
    in_maps = []
    for c in range(NCORES):
        b, g = c // 2, c % 2
        m = dict(packs[g])
        xt = x[b].reshape(NQ, 512, ND, 128).transpose(3, 0, 2, 1)
        m["x"] = np.ascontiguousarray(xt).astype(ml_dtypes.bfloat16)
        m["tri"] = tri
        m["ident"] = ident
        in_maps.append(m)
    return in_maps


def kernel(x, Wq, Wk, Wv, Wo, bo):
    from concourse.bass_utils import run_bass_kernel_spmd

    x = np.asarray(x, np.float32)
    Wq, Wk, Wv = (np.asarray(a, np.float32) for a in (Wq, Wk, Wv))
    Wo = np.asarray(Wo, np.float32)
    bo = np.asarray(bo, np.float32)

    if "nc" not in _CACHE:
        _CACHE["nc"] = _build_program()
    nc = _CACHE["nc"]

    in_maps = _pack_inputs(x, Wq, Wk, Wv, Wo)
    res = run_bass_kernel_spmd(nc, in_maps, list(range(NCORES)))
    _CACHE["last_result"] = res

    out = np.empty((B, T, D), np.float32)
    for b in range(B):
        yt = res.results[2 * b]["yt"] + res.results[2 * b + 1]["yt"]
        out[b] = yt.T + bo
    return out



# revision 16
# speedup vs baseline: 1.1518x; 1.1518x over previous
"""Multi-head causal attention (B=4, T=2048, D=1024, H=16, HS=64) on 8 TRN2
NeuronCores.

Sharding: batch (4-way) x head-group (2-way).  Core c handles batch c//2 and
heads 8*(c%2) .. 8*(c%2)+7.  Each core computes its 8 heads' attention and the
partial output projection Y_T = sum_h Wo_h^T @ O_T_h; the host sums the two
head-group partials per batch, transposes, and adds the output bias.

v2 structure (vs the v1 baseline):
  - V is produced directly in [t, e] layout (lhsT = x^T chunks, rhs = all 8
    heads' Wv columns), which removes all 128 PE transposes.
  - Scores S^T [k, q] contract over e=64, so the two heads of a pair are
    row-packed: head 0 on PE rows 0-63, head 1 on rows 64-127 (tile_position
    auto-derived from base partitions).  Adjacent emission makes the two
    matmuls run concurrently -> ~2x on the S stream.
  - S lands in PSUM as bf16: one 2KB bank holds a [128, 2, 512] slot (two
    k-chunks), so exp runs as [128, ~1024] activations and the whole
    attention pipeline fits in 8 banks: 4 x S (2 heads x 2 slots in flight)
    + 2 x O accumulators + 2 x general matmul banks.
  - The output projection runs as filler inside pair 3's attention: for each
    (dc, qc) all four pair contributions accumulate in one PSUM bank, then a
    single copy + DMA out.  Q/K projections for pair p+1 and the remaining
    V-projection chunks fill pairs 0-2.
  - ScalarE does (almost) only exp; psum evacuations and the softmax
    normalization (1/l broadcast via a DRAM bounce) run on VectorE in bf16.
"""

import numpy as np

B, T, D = 4, 2048, 1024
H, HS = 16, 64
NCORES = 8
NPAIR = 4   # head pairs per core
ND = 8      # 128-wide d chunks
NT = 16     # 128-wide t chunks
NQ = 4      # 512-wide q chunks
NK = 16     # 128-wide k chunks

_CACHE = {}


def _build_program(dbg=False):
    import concourse.bass as bass
    import concourse.tile as tile
    from concourse import bacc, mybir
    from contextlib import ExitStack

    f32 = mybir.dt.float32
    bf16 = mybir.dt.bfloat16
    Exp = mybir.ActivationFunctionType.Exp

    nc = bacc.Bacc("TRN2", target_bir_lowering=False, debug=False)

    x_d = nc.declare_dram_parameter("x", [128, NQ, ND, 512], bf16, isOutput=False)
    wq_d = nc.declare_dram_parameter("wq", [NPAIR, 128, ND, 128], bf16, isOutput=False)
    wk_d = nc.declare_dram_parameter("wk", [NPAIR, 128, ND, 128], bf16, isOutput=False)
    wv_d = nc.declare_dram_parameter("wv", [128, ND, 512], bf16, isOutput=False)
    wo_d = nc.declare_dram_parameter("wo", [128, NPAIR, ND, 128], bf16, isOutput=False)
    tri_d = nc.declare_dram_parameter("tri", [128, 128], bf16, isOutput=False)
    yt_d = nc.declare_dram_parameter("yt", [D, T], f32, isOutput=True)
    if dbg:
        qt_dbg = nc.declare_dram_parameter("qt_dbg", [128, NPAIR, T], bf16, isOutput=True)
        kt_dbg = nc.declare_dram_parameter("kt_dbg", [128, NPAIR, T], bf16, isOutput=True)
        va_dbg = nc.declare_dram_parameter("va_dbg", [128, NT, 8, 65], bf16, isOutput=True)
        ot_dbg = nc.declare_dram_parameter("ot_dbg", [128, NPAIR, T], bf16, isOutput=True)
        pt_dbg = nc.declare_dram_parameter("pt_dbg", [4, 128, 2, 512], bf16, isOutput=True)
        oc_dbg = nc.declare_dram_parameter("oc_dbg", [2, 64, 512], f32, isOutput=True)
        rl_dbg = nc.declare_dram_parameter("rl_dbg", [2, 1, 512], f32, isOutput=True)
        lb_dbg = nc.declare_dram_parameter("lb_dbg", [2, 64, 512], f32, isOutput=True)
        dbg_state = {"pt": [], "norm": []}

    with tile.TileContext(nc) as tc, ExitStack() as top:
        const = top.enter_context(tc.tile_pool(name="const", bufs=1))
        # tri2[:, h, :] = upper-triangular causal mask, replicated per head so
        # one DVE mul masks both heads of a pair
        tri2 = const.tile([128, 2, 128], bf16, name="tri2")
        nc.sync.dma_start(out=tri2[:, 0, :], in_=tri_d[:, :])
        nc.sync.dma_start(out=tri2[:, 1, :], in_=tri_d[:, :])
        # touch Exp early so the ~2.7us ACT table load overlaps phase A
        scr = const.tile([1, 8], bf16, name="scr")
        nc.scalar.activation(out=scr, in_=tri2[0:1, 0, 0:8], func=Exp, scale=1.0)

        big = top.enter_context(tc.tile_pool(name="big", bufs=1))
        # vaug[:, c, h, 0:64] = V[t=c*128..+128, e=h*64..+64]; col 64 = ones
        vaug = big.tile([128, NT, 8, 65], bf16, name="vaug")
        nc.vector.memset(vaug[:, :, :, 64:65], 1.0)

        xtp = top.enter_context(tc.tile_pool(name="xtp", bufs=1))
        xt = xtp.tile([128, NQ, ND, 512], bf16, name="xt")
        wvp = top.enter_context(tc.tile_pool(name="wvp", bufs=1))
        wv_sb = wvp.tile([128, ND, 512], bf16, name="wv_sb")
        qkp = top.enter_context(tc.tile_pool(name="qkp", bufs=1))
        qt = qkp.tile([128, NPAIR, T], bf16, name="qt")
        kt = qkp.tile([128, NPAIR, T], bf16, name="kt")
        otn_p = top.enter_context(tc.tile_pool(name="otn_p", bufs=1))
        otn = otn_p.tile([128, NPAIR, T], bf16, name="otn")
        pwo = top.enter_context(tc.tile_pool(name="pwo", bufs=1))
        wo_sb = pwo.tile([128, NPAIR, ND, 128], bf16, name="wo_sb")

        pw = top.enter_context(tc.tile_pool(name="pw", bufs=4))
        ptp = top.enter_context(tc.tile_pool(name="ptp", bufs=4))
        ocp = top.enter_context(tc.tile_pool(name="ocp", bufs=2))
        rcp = top.enter_context(tc.tile_pool(name="rcp", bufs=2))
        lbp = top.enter_context(tc.tile_pool(name="lbp", bufs=2))
        pyt = top.enter_context(tc.tile_pool(name="pyt", bufs=3))
        drp = top.enter_context(tc.tile_pool(name="drp", bufs=4, space="DRAM"))

        # PSUM budget: psS 2x2 banks + psO 2 + psM 2 = 8
        psS = top.enter_context(tc.tile_pool(name="psS", bufs=2, space="PSUM"))
        psO = top.enter_context(tc.tile_pool(name="psO", bufs=2, space="PSUM"))
        psM = top.enter_context(tc.tile_pool(name="psM", bufs=2, space="PSUM"))

        # ---- DMA everything up front (queues overlap compute) --------------
        for t4 in range(NQ):
            nc.sync.dma_start(out=xt[:, t4, :, :], in_=x_d[:, t4, :, :])
        nc.sync.dma_start(out=wv_sb, in_=wv_d[:, :, :])
        wq_sbs = [None] * NPAIR
        wk_sbs = [None] * NPAIR

        def dma_w(wdram, p, kind):
            w_sb = pw.tile([128, ND, 128], bf16, tag="w", name=f"w_{kind}{p}")
            nc.sync.dma_start(out=w_sb, in_=wdram[p])
            return w_sb

        wq_sbs[0] = dma_w(wq_d, 0, "q")
        wk_sbs[0] = dma_w(wk_d, 0, "k")
        nc.sync.dma_start(out=wo_sb, in_=wo_d[:, :, :, :])

        # ---- building blocks ----------------------------------------------
        def qk_proj_mms(ps, w_sb, t4, dc_lo, dc_hi):
            for dc in range(dc_lo, dc_hi):
                nc.tensor.matmul(
                    ps, w_sb[:, dc, :], xt[:, t4, dc, :],
                    start=(dc == 0), stop=(dc == ND - 1),
                )

        def v_proj(tc_):
            """V[t, e] for one 128-token chunk, all 8 heads at once."""
            t4, sub = tc_ // 4, tc_ % 4
            ps = psM.tile([128, 512], f32, tag="mm", name="psv")
            for dc in range(ND):
                nc.tensor.matmul(
                    ps,
                    xt[:, t4, dc, sub * 128:(sub + 1) * 128],
                    wv_sb[:, dc, :],
                    start=(dc == 0), stop=(dc == ND - 1),
                )
            nc.vector.tensor_copy(out=vaug[:, tc_, :, 0:64], in_=ps)

        # ---- Phase A: Q/K pair 0 + V chunks 0-3 ----------------------------
        for w_sb, dest in ((wq_sbs[0], qt), (wk_sbs[0], kt)):
            for t4 in range(NQ):
                ps = psM.tile([128, 512], f32, tag="mm", name="psqk")
                qk_proj_mms(ps, w_sb, t4, 0, ND)
                nc.scalar.copy(out=dest[:, 0, t4 * 512:(t4 + 1) * 512], in_=ps)
        for tc_ in range(4):
            v_proj(tc_)

        # Deferred softmax-normalize multiplies: the 1/l partition-broadcast
        # rides a DRAM bounce; emitting the dependent DVE mul immediately
        # would head-of-line-block the in-order VectorE queue.  Each group
        # queues its two muls here; the next group flushes them (the DMA has
        # completed by then).
        pending_norm = []

        def flush_norm():
            while pending_norm:
                pending_norm.pop(0)()

        # ---- attention group for one head pair, one q-chunk ----------------
        def attn_group(p, j, filler):
            ncc = 4 * (j + 1)
            jw = j * 512
            flush_norm()
            po = [psO.tile([65, 512], f32, tag="O", name=f"po{h}")
                  for h in range(2)]
            pts = {}

            def off_of(c):
                sub = c - 4 * j
                return sub * 128 if 0 <= sub < 4 else 0

            def emit_s(c):
                # ps[:, h, :] spans two PSUM banks: the row-packed head
                # matmuls write different banks, one exp call reads both
                off = off_of(c)
                ps = psS.tile([128, 2, 512], f32, tag="S", name="ps")
                pt = ptp.tile([128, 2, 512], bf16, tag="pt", name="pt")
                for h in range(2):
                    e0 = h * 64
                    nc.tensor.matmul(
                        ps[:, h, off:],
                        kt[e0:e0 + 64, p, c * 128:(c + 1) * 128],
                        qt[e0:e0 + 64, p, jw + off:jw + 512],
                        start=True, stop=True,
                    )
                nc.scalar.activation(out=pt[:, :, off:], in_=ps[:, :, off:],
                                     func=Exp, scale=0.125)
                sub = c - 4 * j
                if 0 <= sub < 4:
                    nc.vector.tensor_mul(
                        pt[:, :, sub * 128:(sub + 1) * 128],
                        pt[:, :, sub * 128:(sub + 1) * 128],
                        tri2,
                    )
                if dbg and p == 0 and j == 0:
                    nc.sync.dma_start(out=pt_dbg[c], in_=pt)
                pts[c] = pt

            def emit_v(c):
                pt = pts.pop(c)
                off = off_of(c)
                for h in range(2):
                    nc.tensor.matmul(
                        po[h][:, off:],
                        vaug[:, c, 2 * p + h, :],
                        pt[:, h, off:],
                        start=(c == 0), stop=(c == ncc - 1),
                    )

            emit_s(0)
            if ncc > 1:
                emit_s(1)
            for c in range(ncc):
                if c + 2 < ncc:
                    emit_s(c + 2)
                if c % 2 == 0:
                    filler()
                emit_v(c)

            # normalize: otn[e, q] = O_T[e, q] / l[q] on DVE;
            # the final mul is deferred until the broadcast DMA has landed
            for h in range(2):
                oc = ocp.tile([64, 512], f32, tag="oc", name="oc")
                nc.vector.tensor_copy(out=oc, in_=po[h][0:64, :])
                rlc = rcp.tile([1, 512], f32, tag="rlc", name="rlc")
                nc.vector.tensor_copy(out=rlc, in_=po[h][64:65, :])
                rl = rcp.tile([1, 512], f32, tag="rl", name="rl")
                nc.vector.reciprocal_approx_fast(rl, rlc)
                rd = drp.tile([1, 512], f32, tag="rd", name="rd")
                nc.sync.dma_start(out=rd, in_=rl)
                lb = lbp.tile([64, 512], f32, tag="lb", name="lb")
                nc.sync.dma_start(out=lb, in_=rd[0:1, :].partition_broadcast(64))
                e0 = h * 64

                def norm_mul(oc=oc, lb=lb, e0=e0, p=p, jw=jw):
                    nc.vector.tensor_mul(
                        otn[e0:e0 + 64, p, jw:jw + 512], oc, lb
                    )
                pending_norm.append(norm_mul)
                if dbg and p == 0 and j == 0:
                    nc.sync.dma_start(out=oc_dbg[h], in_=oc)
                    nc.sync.dma_start(out=rl_dbg[h], in_=rl)
                    nc.sync.dma_start(out=lb_dbg[h], in_=lb)

        # ---- filler units ---------------------------------------------------
        def mk_qk_unit(w_sb, dest, p, t4, dc_lo, dc_hi, state):
            def emit():
                if dc_lo == 0:
                    state["ps"] = psM.tile([128, 512], f32, tag="mm", name="psf")
                qk_proj_mms(state["ps"], w_sb, t4, dc_lo, dc_hi)
                if dc_hi == ND:
                    nc.vector.tensor_copy(
                        out=dest[:, p, t4 * 512:(t4 + 1) * 512],
                        in_=state["ps"])
            return emit

        def mk_out_unit(dc, qc):
            def emit():
                py = psM.tile([128, 512], f32, tag="mm", name="pyo")
                for pp in range(NPAIR):
                    nc.tensor.matmul(
                        py,
                        wo_sb[:, pp, dc, :],
                        otn[:, pp, qc * 512:(qc + 1) * 512],
                        start=(pp == 0), stop=(pp == NPAIR - 1),
                    )
                yt_sb = pyt.tile([128, 512], f32, tag="yt", name="yt_f")
                nc.vector.tensor_copy(out=yt_sb, in_=py)
                nc.sync.dma_start(
                    out=yt_d[dc * 128:(dc + 1) * 128,
                             qc * 512:(qc + 1) * 512],
                    in_=yt_sb,
                )
            return emit

        # ---- Phase B: four pair phases -------------------------------------
        for p in range(NPAIR):
            fill = []
            if p == 0:
                # remaining V chunks first (rate-2 filler covers readiness:
                # j=1 needs tc<8 after j=0's fillers, etc.), then QK pair 1
                for tc_ in range(4, NT):
                    fill.append(lambda tc_=tc_: v_proj(tc_))
            if p < NPAIR - 1:
                wq_sbs[p + 1] = dma_w(wq_d, p + 1, "q")
                wk_sbs[p + 1] = dma_w(wk_d, p + 1, "k")
                for w_sb, dest in ((wq_sbs[p + 1], qt), (wk_sbs[p + 1], kt)):
                    for t4 in range(NQ):
                        state = {}
                        for dc_lo in (0, 4):
                            fill.append(mk_qk_unit(w_sb, dest, p + 1, t4,
                                                   dc_lo, dc_lo + 4, state))

            def filler(fill=fill):
                n = 2 if fill and len(fill) > 8 else 1
                for _ in range(n):
                    if fill:
                        fill.pop(0)()

            if p < NPAIR - 1:
                for j in range(NQ):
                    attn_group(p, j, filler)
            else:
                # pair 3: out-projection as filler, staggered by q readiness
                for j in range(NQ):
                    if j >= 1:
                        qc = j - 1
                        for dc in range(ND):
                            fill.append(mk_out_unit(dc, qc))
                    attn_group(p, j, filler)
            while fill:
                fill.pop(0)()

        # tail: last q-chunk of the output projection
        flush_norm()
        for dc in range(ND):
            mk_out_unit(dc, NQ - 1)()

        if dbg:
            nc.sync.dma_start(out=qt_dbg[:, :, :], in_=qt)
            nc.sync.dma_start(out=kt_dbg[:, :, :], in_=kt)
            nc.sync.dma_start(out=va_dbg[:, :, :, :], in_=vaug)
            nc.sync.dma_start(out=ot_dbg[:, :, :], in_=otn)

    nc.compile()
    return nc


def _pack_inputs(x, Wq, Wk, Wv, Wo):
    """Per-core input maps. Core c: batch c//2, head group c%2."""
    import ml_dtypes

    tri = np.triu(np.ones((128, 128), np.float32)).astype(ml_dtypes.bfloat16)

    def pack_w(W, g):
        # [NPAIR, 128(d_local), ND, 128(e2)]
        out = np.empty((NPAIR, 128, ND, 128), np.float32)
        for p in range(NPAIR):
            h1 = 8 * g + 2 * p
            r = W[[h1, h1 + 1]].transpose(1, 0, 2).reshape(D, 128)  # [d, e2]
            out[p] = r.reshape(ND, 128, 128).transpose(1, 0, 2)
        return np.ascontiguousarray(out).astype(ml_dtypes.bfloat16)

    def pack_wv(W, g):
        # [128(d within chunk), ND, 512(e = h*64+hs over 8 heads)]
        r = W[8 * g:8 * g + 8].transpose(1, 0, 2).reshape(D, 512)  # [d, e]
        out = r.reshape(ND, 128, 512).transpose(1, 0, 2)
        return np.ascontiguousarray(out).astype(ml_dtypes.bfloat16)

    def pack_wo(Wo, g):
        # [128(e2), NPAIR, ND, 128(d)]
        out = np.empty((128, NPAIR, ND, 128), np.float32)
        for p in range(NPAIR):
            r0 = (8 * g + 2 * p) * 64
            out[:, p] = Wo[r0:r0 + 128].reshape(128, ND, 128)
        return np.ascontiguousarray(out).astype(ml_dtypes.bfloat16)

    packs = {}
    for g in range(2):
        packs[g] = dict(
            wq=pack_w(Wq, g), wk=pack_w(Wk, g), wv=pack_wv(Wv, g),
            wo=pack_wo(Wo, g),
        )
    in_maps = []
    for c in range(NCORES):
        b, g = c // 2, c % 2
        m = dict(packs[g])
        xt = x[b].reshape(NQ, 512, ND, 128).transpose(3, 0, 2, 1)
        m["x"] = np.ascontiguousarray(xt).astype(ml_dtypes.bfloat16)
        m["tri"] = tri
        in_maps.append(m)
    return in_maps


def kernel(x, Wq, Wk, Wv, Wo, bo):
    from concourse.bass_utils import run_bass_kernel_spmd

    x = np.asarray(x, np.float32)
    Wq, Wk, Wv = (np.asarray(a, np.float32) for a in (Wq, Wk, Wv))
    Wo = np.asarray(Wo, np.float32)
    bo = np.asarray(bo, np.float32)

    if "nc" not in _CACHE:
        _CACHE["nc"] = _build_program()
    nc = _CACHE["nc"]

    in_maps = _pack_inputs(x, Wq, Wk, Wv, Wo)
    res = run_bass_kernel_spmd(nc, in_maps, list(range(NCORES)))
    _CACHE["last_result"] = res

    out = np.empty((B, T, D), np.float32)
    for b in range(B):
        yt = res.results[2 * b]["yt"] + res.results[2 * b + 1]["yt"]
        out[b] = yt.T + bo
    return out


# revision 17
# speedup vs baseline: 1.2021x; 1.0437x over previous
"""Multi-head causal attention (B=4, T=2048, D=1024, H=16, HS=64) on 8 TRN2
NeuronCores.

Sharding: batch (4-way) x head-group (2-way).  Core c handles batch c//2 and
heads 8*(c%2) .. 8*(c%2)+7.  Each core computes its 8 heads' attention and the
partial output projection Y_T = sum_h Wo_h^T @ O_T_h; the host sums the two
head-group partials per batch, transposes, and adds the output bias.

v2 structure (vs the v1 baseline):
  - V is produced directly in [t, e] layout (lhsT = x^T chunks, rhs = all 8
    heads' Wv columns), which removes all 128 PE transposes.
  - Scores S^T [k, q] contract over e=64, so the two heads of a pair are
    row-packed: head 0 on PE rows 0-63, head 1 on rows 64-127 (tile_position
    auto-derived from base partitions).  Adjacent emission makes the two
    matmuls run concurrently -> ~2x on the S stream.
  - S lands in PSUM as bf16: one 2KB bank holds a [128, 2, 512] slot (two
    k-chunks), so exp runs as [128, ~1024] activations and the whole
    attention pipeline fits in 8 banks: 4 x S (2 heads x 2 slots in flight)
    + 2 x O accumulators + 2 x general matmul banks.
  - The output projection runs as filler inside pair 3's attention: for each
    (dc, qc) all four pair contributions accumulate in one PSUM bank, then a
    single copy + DMA out.  Q/K projections for pair p+1 and the remaining
    V-projection chunks fill pairs 0-2.
  - ScalarE does (almost) only exp; psum evacuations and the softmax
    normalization (1/l broadcast via a DRAM bounce) run on VectorE in bf16.
"""

import numpy as np

B, T, D = 4, 2048, 1024
H, HS = 16, 64
NCORES = 8
NPAIR = 4   # head pairs per core
ND = 8      # 128-wide d chunks
NT = 16     # 128-wide t chunks
NQ = 4      # 512-wide q chunks
NK = 16     # 128-wide k chunks

_CACHE = {}


def _build_program(dbg=False):
    import concourse.bass as bass
    import concourse.tile as tile
    from concourse import bacc, mybir
    from contextlib import ExitStack

    f32 = mybir.dt.float32
    bf16 = mybir.dt.bfloat16
    Exp = mybir.ActivationFunctionType.Exp

    nc = bacc.Bacc("TRN2", target_bir_lowering=False, debug=False)

    x_d = nc.declare_dram_parameter("x", [128, NQ, ND, 512], bf16, isOutput=False)
    wq_d = nc.declare_dram_parameter("wq", [NPAIR, 128, ND, 128], bf16, isOutput=False)
    wk_d = nc.declare_dram_parameter("wk", [NPAIR, 128, ND, 128], bf16, isOutput=False)
    wv_d = nc.declare_dram_parameter("wv", [128, ND, 512], bf16, isOutput=False)
    wo_d = nc.declare_dram_parameter("wo", [128, NPAIR, ND, 128], bf16, isOutput=False)
    tri_d = nc.declare_dram_parameter("tri", [128, 128], bf16, isOutput=False)
    yt_d = nc.declare_dram_parameter("yt", [D, T], f32, isOutput=True)
    if dbg:
        qt_dbg = nc.declare_dram_parameter("qt_dbg", [128, NPAIR, T], bf16, isOutput=True)
        kt_dbg = nc.declare_dram_parameter("kt_dbg", [128, NPAIR, T], bf16, isOutput=True)
        va_dbg = nc.declare_dram_parameter("va_dbg", [128, NT, 8, 65], bf16, isOutput=True)
        ot_dbg = nc.declare_dram_parameter("ot_dbg", [128, NPAIR, T], bf16, isOutput=True)
        pt_dbg = nc.declare_dram_parameter("pt_dbg", [4, 128, 2, 512], bf16, isOutput=True)
        oc_dbg = nc.declare_dram_parameter("oc_dbg", [2, 64, 512], f32, isOutput=True)
        rl_dbg = nc.declare_dram_parameter("rl_dbg", [2, 1, 512], f32, isOutput=True)
        lb_dbg = nc.declare_dram_parameter("lb_dbg", [2, 64, 512], f32, isOutput=True)
        dbg_state = {"pt": [], "norm": []}

    with tile.TileContext(nc) as tc, ExitStack() as top:
        const = top.enter_context(tc.tile_pool(name="const", bufs=1))
        # tri2[:, h, :] = upper-triangular causal mask, replicated per head so
        # one DVE mul masks both heads of a pair
        tri2 = const.tile([128, 2, 128], bf16, name="tri2")
        nc.sync.dma_start(out=tri2[:, 0, :], in_=tri_d[:, :])
        nc.sync.dma_start(out=tri2[:, 1, :], in_=tri_d[:, :])
        # touch Exp early so the ~2.7us ACT table load overlaps phase A
        scr = const.tile([1, 8], bf16, name="scr")
        nc.scalar.activation(out=scr, in_=tri2[0:1, 0, 0:8], func=Exp, scale=1.0)

        big = top.enter_context(tc.tile_pool(name="big", bufs=1))
        # vaug[:, c, h, 0:64] = V[t=c*128..+128, e=h*64..+64]; col 64 = ones
        vaug = big.tile([128, NT, 8, 65], bf16, name="vaug")
        nc.vector.memset(vaug[:, :, :, 64:65], 1.0)

        xtp = top.enter_context(tc.tile_pool(name="xtp", bufs=1))
        xt = xtp.tile([128, NQ, ND, 512], bf16, name="xt")
        wvp = top.enter_context(tc.tile_pool(name="wvp", bufs=1))
        wv_sb = wvp.tile([128, ND, 512], bf16, name="wv_sb")
        qkp = top.enter_context(tc.tile_pool(name="qkp", bufs=1))
        qt = qkp.tile([128, NPAIR, T], bf16, name="qt")
        kt = qkp.tile([128, NPAIR, T], bf16, name="kt")
        otn_p = top.enter_context(tc.tile_pool(name="otn_p", bufs=1))
        otn = otn_p.tile([128, NPAIR, T], bf16, name="otn")
        pwo = top.enter_context(tc.tile_pool(name="pwo", bufs=1))
        wo_sb = pwo.tile([128, NPAIR, ND, 128], bf16, name="wo_sb")

        pw = top.enter_context(tc.tile_pool(name="pw", bufs=4))
        ptp = top.enter_context(tc.tile_pool(name="ptp", bufs=4))
        ocp = top.enter_context(tc.tile_pool(name="ocp", bufs=2))
        rcp = top.enter_context(tc.tile_pool(name="rcp", bufs=2))
        lbp = top.enter_context(tc.tile_pool(name="lbp", bufs=2))
        pyt = top.enter_context(tc.tile_pool(name="pyt", bufs=3))
        drp = top.enter_context(tc.tile_pool(name="drp", bufs=4, space="DRAM"))

        # PSUM budget: psS 2x2 banks + psO 2 + psM 2 = 8
        psS = top.enter_context(tc.tile_pool(name="psS", bufs=2, space="PSUM"))
        psO = top.enter_context(tc.tile_pool(name="psO", bufs=2, space="PSUM"))
        psM = top.enter_context(tc.tile_pool(name="psM", bufs=2, space="PSUM"))

        # ---- DMA in first-use order (startup is DMA-latency-bound) ---------
        wq_sbs = [None] * NPAIR
        wk_sbs = [None] * NPAIR

        def dma_w(wdram, p, kind):
            w_sb = pw.tile([128, ND, 128], bf16, tag="w", name=f"w_{kind}{p}")
            nc.sync.dma_start(out=w_sb, in_=wdram[p])
            return w_sb

        wq_sbs[0] = dma_w(wq_d, 0, "q")
        wk_sbs[0] = dma_w(wk_d, 0, "k")
        nc.sync.dma_start(out=xt[:, 0, :, :], in_=x_d[:, 0, :, :])
        nc.sync.dma_start(out=wv_sb, in_=wv_d[:, :, :])
        for t4 in range(1, NQ):
            nc.sync.dma_start(out=xt[:, t4, :, :], in_=x_d[:, t4, :, :])
        nc.sync.dma_start(out=wo_sb, in_=wo_d[:, :, :, :])

        # ---- building blocks ----------------------------------------------
        def qk_proj_mms(ps, w_sb, t4, dc_lo, dc_hi):
            for dc in range(dc_lo, dc_hi):
                nc.tensor.matmul(
                    ps, w_sb[:, dc, :], xt[:, t4, dc, :],
                    start=(dc == 0), stop=(dc == ND - 1),
                )

        def v_proj(tc_):
            """V[t, e] for one 128-token chunk, all 8 heads at once."""
            t4, sub = tc_ // 4, tc_ % 4
            ps = psM.tile([128, 512], f32, tag="mm", name="psv")
            for dc in range(ND):
                nc.tensor.matmul(
                    ps,
                    xt[:, t4, dc, sub * 128:(sub + 1) * 128],
                    wv_sb[:, dc, :],
                    start=(dc == 0), stop=(dc == ND - 1),
                )
            nc.vector.tensor_copy(out=vaug[:, tc_, :, 0:64], in_=ps)

        # ---- Phase A: Q/K pair 0 + V chunks 0-3 (t4=0 work first) ----------
        def qk0(w_sb, dest, t4):
            ps = psM.tile([128, 512], f32, tag="mm", name="psqk")
            qk_proj_mms(ps, w_sb, t4, 0, ND)
            nc.scalar.copy(out=dest[:, 0, t4 * 512:(t4 + 1) * 512], in_=ps)

        qk0(wq_sbs[0], qt, 0)
        qk0(wk_sbs[0], kt, 0)
        for tc_ in range(4):
            v_proj(tc_)
        for t4 in range(1, NQ):
            qk0(wq_sbs[0], qt, t4)
            qk0(wk_sbs[0], kt, t4)

        # Deferred softmax-normalize multiplies: the 1/l partition-broadcast
        # rides a DRAM bounce; emitting the dependent DVE mul immediately
        # would head-of-line-block the in-order VectorE queue.  Each group
        # queues its two muls here; the next group flushes them (the DMA has
        # completed by then).
        pending_norm = []

        def flush_norm():
            while pending_norm:
                pending_norm.pop(0)()

        # ---- attention group for one head pair, one q-chunk ----------------
        def attn_group(p, j, filler):
            ncc = 4 * (j + 1)
            jw = j * 512
            flush_norm()
            po = [psO.tile([65, 512], f32, tag="O", name=f"po{h}")
                  for h in range(2)]
            pts = {}

            def off_of(c):
                sub = c - 4 * j
                return sub * 128 if 0 <= sub < 4 else 0

            def emit_s(c):
                # ps[:, h, :] spans two PSUM banks: the row-packed head
                # matmuls write different banks, one exp call reads both
                off = off_of(c)
                ps = psS.tile([128, 2, 512], f32, tag="S", name="ps")
                pt = ptp.tile([128, 2, 512], bf16, tag="pt", name="pt")
                for h in range(2):
                    e0 = h * 64
                    nc.tensor.matmul(
                        ps[:, h, off:],
                        kt[e0:e0 + 64, p, c * 128:(c + 1) * 128],
                        qt[e0:e0 + 64, p, jw + off:jw + 512],
                        start=True, stop=True,
                    )
                nc.scalar.activation(out=pt[:, :, off:], in_=ps[:, :, off:],
                                     func=Exp, scale=0.125)
                sub = c - 4 * j
                if 0 <= sub < 4:
                    nc.vector.tensor_mul(
                        pt[:, :, sub * 128:(sub + 1) * 128],
                        pt[:, :, sub * 128:(sub + 1) * 128],
                        tri2,
                    )
                if dbg and p == 0 and j == 0:
                    nc.sync.dma_start(out=pt_dbg[c], in_=pt)
                pts[c] = pt

            def emit_v(c):
                pt = pts.pop(c)
                off = off_of(c)
                for h in range(2):
                    nc.tensor.matmul(
                        po[h][:, off:],
                        vaug[:, c, 2 * p + h, :],
                        pt[:, h, off:],
                        start=(c == 0), stop=(c == ncc - 1),
                    )

            emit_s(0)
            if ncc > 1:
                emit_s(1)
            for c in range(ncc):
                if c + 2 < ncc:
                    emit_s(c + 2)
                if c % 2 == 0:
                    filler()
                emit_v(c)

            # normalize: otn[e, q] = O_T[e, q] / l[q] on DVE;
            # the final mul is deferred until the broadcast DMA has landed
            for h in range(2):
                oc = ocp.tile([64, 512], f32, tag="oc", name="oc")
                nc.vector.tensor_copy(out=oc, in_=po[h][0:64, :])
                rlc = rcp.tile([1, 512], f32, tag="rlc", name="rlc")
                nc.vector.tensor_copy(out=rlc, in_=po[h][64:65, :])
                rl = rcp.tile([1, 512], f32, tag="rl", name="rl")
                nc.vector.reciprocal_approx_fast(rl, rlc)
                lb = lbp.tile([64, 512], f32, tag="lb", name="lb")
                nc.gpsimd.partition_broadcast(lb, rl)
                e0 = h * 64

                def norm_mul(oc=oc, lb=lb, e0=e0, p=p, jw=jw):
                    nc.vector.tensor_mul(
                        otn[e0:e0 + 64, p, jw:jw + 512], oc, lb
                    )
                pending_norm.append(norm_mul)
                if dbg and p == 0 and j == 0:
                    nc.sync.dma_start(out=oc_dbg[h], in_=oc)
                    nc.sync.dma_start(out=rl_dbg[h], in_=rl)
                    nc.sync.dma_start(out=lb_dbg[h], in_=lb)

        # ---- filler units ---------------------------------------------------
        def mk_qk_unit(w_sb, dest, p, t4, dc_lo, dc_hi, state):
            def emit():
                if dc_lo == 0:
                    state["ps"] = psM.tile([128, 512], f32, tag="mm", name="psf")
                qk_proj_mms(state["ps"], w_sb, t4, dc_lo, dc_hi)
                if dc_hi == ND:
                    nc.vector.tensor_copy(
                        out=dest[:, p, t4 * 512:(t4 + 1) * 512],
                        in_=state["ps"])
            return emit

        def mk_out_unit(dc, qc):
            def emit():
                py = psM.tile([128, 512], f32, tag="mm", name="pyo")
                for pp in range(NPAIR):
                    nc.tensor.matmul(
                        py,
                        wo_sb[:, pp, dc, :],
                        otn[:, pp, qc * 512:(qc + 1) * 512],
                        start=(pp == 0), stop=(pp == NPAIR - 1),
                    )
                yt_sb = pyt.tile([128, 512], f32, tag="yt", name="yt_f")
                nc.vector.tensor_copy(out=yt_sb, in_=py)
                nc.sync.dma_start(
                    out=yt_d[dc * 128:(dc + 1) * 128,
                             qc * 512:(qc + 1) * 512],
                    in_=yt_sb,
                )
            return emit

        # ---- Phase B: four pair phases -------------------------------------
        for p in range(NPAIR):
            fill = []
            if p == 0:
                # remaining V chunks first (rate-2 filler covers readiness:
                # j=1 needs tc<8 after j=0's fillers, etc.), then QK pair 1
                for tc_ in range(4, NT):
                    fill.append(lambda tc_=tc_: v_proj(tc_))
            if p < NPAIR - 1:
                wq_sbs[p + 1] = dma_w(wq_d, p + 1, "q")
                wk_sbs[p + 1] = dma_w(wk_d, p + 1, "k")
                for w_sb, dest in ((wq_sbs[p + 1], qt), (wk_sbs[p + 1], kt)):
                    for t4 in range(NQ):
                        state = {}
                        for dc_lo in (0, 4):
                            fill.append(mk_qk_unit(w_sb, dest, p + 1, t4,
                                                   dc_lo, dc_lo + 4, state))

            def filler(fill=fill):
                n = 2 if fill and len(fill) > 8 else 1
                for _ in range(n):
                    if fill:
                        fill.pop(0)()

            if p < NPAIR - 1:
                for j in range(NQ):
                    attn_group(p, j, filler)
            else:
                # pair 3: j descending, so the tail waits only on the
                # shortest group's normalize; out-units follow completions
                for j in (3, 2, 1, 0):
                    if j < 3:
                        qc = j + 1
                        for dc in range(ND):
                            fill.append(mk_out_unit(dc, qc))
                    attn_group(p, j, filler)
            while fill:
                fill.pop(0)()

        # tail: q-chunk 0 of the output projection
        flush_norm()
        for dc in range(ND):
            mk_out_unit(dc, 0)()

        if dbg:
            nc.sync.dma_start(out=qt_dbg[:, :, :], in_=qt)
            nc.sync.dma_start(out=kt_dbg[:, :, :], in_=kt)
            nc.sync.dma_start(out=va_dbg[:, :, :, :], in_=vaug)
            nc.sync.dma_start(out=ot_dbg[:, :, :], in_=otn)

    nc.compile()
    return nc


def _pack_inputs(x, Wq, Wk, Wv, Wo):
    """Per-core input maps. Core c: batch c//2, head group c%2."""
    import ml_dtypes

    tri = np.triu(np.ones((128, 128), np.float32)).astype(ml_dtypes.bfloat16)

    def pack_w(W, g):
        # [NPAIR, 128(d_local), ND, 128(e2)]
        out = np.empty((NPAIR, 128, ND, 128), np.float32)
        for p in range(NPAIR):
            h1 = 8 * g + 2 * p
            r = W[[h1, h1 + 1]].transpose(1, 0, 2).reshape(D, 128)  # [d, e2]
            out[p] = r.reshape(ND, 128, 128).transpose(1, 0, 2)
        return np.ascontiguousarray(out).astype(ml_dtypes.bfloat16)

    def pack_wv(W, g):
        # [128(d within chunk), ND, 512(e = h*64+hs over 8 heads)]
        r = W[8 * g:8 * g + 8].transpose(1, 0, 2).reshape(D, 512)  # [d, e]
        out = r.reshape(ND, 128, 512).transpose(1, 0, 2)
        return np.ascontiguousarray(out).astype(ml_dtypes.bfloat16)

    def pack_wo(Wo, g):
        # [128(e2), NPAIR, ND, 128(d)]
        out = np.empty((128, NPAIR, ND, 128), np.float32)
        for p in range(NPAIR):
            r0 = (8 * g + 2 * p) * 64
            out[:, p] = Wo[r0:r0 + 128].reshape(128, ND, 128)
        return np.ascontiguousarray(out).astype(ml_dtypes.bfloat16)

    packs = {}
    for g in range(2):
        packs[g] = dict(
            wq=pack_w(Wq, g), wk=pack_w(Wk, g), wv=pack_wv(Wv, g),
            wo=pack_wo(Wo, g),
        )
    in_maps = []
    for c in range(NCORES):
        b, g = c // 2, c % 2
        m = dict(packs[g])
        xt = x[b].reshape(NQ, 512, ND, 128).transpose(3, 0, 2, 1)
        m["x"] = np.ascontiguousarray(xt).astype(ml_dtypes.bfloat16)
        m["tri"] = tri
        in_maps.append(m)
    return in_maps


def kernel(x, Wq, Wk, Wv, Wo, bo):
    from concourse.bass_utils import run_bass_kernel_spmd

    x = np.asarray(x, np.float32)
    Wq, Wk, Wv = (np.asarray(a, np.float32) for a in (Wq, Wk, Wv))
    Wo = np.asarray(Wo, np.float32)
    bo = np.asarray(bo, np.float32)

    if "nc" not in _CACHE:
        _CACHE["nc"] = _build_program()
    nc = _CACHE["nc"]

    in_maps = _pack_inputs(x, Wq, Wk, Wv, Wo)
    res = run_bass_kernel_spmd(nc, in_maps, list(range(NCORES)))
    _CACHE["last_result"] = res

    out = np.empty((B, T, D), np.float32)
    for b in range(B):
        yt = res.results[2 * b]["yt"] + res.results[2 * b + 1]["yt"]
        out[b] = yt.T + bo
    return out
